# revision 10
# baseline (speedup 1.0000x reference)
"""Gumbel-softmax palette quantization on 8 TRN2 NeuronCores.

Math (per batch b, pixel p, palette entry k, temperature T):
    gumbel = -ln(-ln(u + eps) + eps)
    probs  = softmax((img + gumbel) / T, axis=k)
    out    = probs @ palette                          # [pix, 4]

T == 1 fast path used here (setup_inputs always has T=1):
    exp(img + gumbel) = exp(img) / (-ln u)
and softmax ratios are invariant to a common per-pixel scale/sign, so with
    t = ln(1 - v)        # v = fp16(1-u) host-encoded, t = ln(u) < 0
    x = exp(img)         # ACT Exp, bf16
    e = x * approx(1/t)  # single fused custom-DVE op (NOT-seed + 1 Newton),
                         # e < 0; sign cancels in the final ratio
    out[p, c] = (e @ [pal|1])[p, c] / (e @ [pal|1])[p, 4]

Why v = 1-u: ln(u) for u->1 is where all the gumbel weight lives, and
fp16(u) destroys 1-u there. 1-u is exact in fp32 for u >= 0.5 (Sterbenz), so
fp16(1-u) keeps ~2^-11 relative precision on t in the tail. The device
computes t = Ln(-v + 1.0) via ACT's free affine - identical instruction cost.

Engine budget per core (16.7M elements):
    ACT  : Ln pass + Exp pass            ~221 us  <- bottleneck (1 elem/cyc/lane)
    DMA  : 2 x 32MiB fp16 in + 1MiB out  ~190 us
    DVE  : fused divide (1x) + epilogue  ~150 us
    PE   : 1024 matmuls [128k,128p]@[128k,5] ~70 us

Sharding: data-parallel over batch, 1 batch per core (b=8, 8 cores).

Layout: host pre-transposes img/noise to k-major [tile, khalf, 128, FT] so
the ACT Exp output is directly the matmul lhsT (k on partitions) - no
device transposes at all. Output written as [tile, 128, block, 4];
host untangles.
"""

import numpy as np
import ml_dtypes

B, H, W, K, C = 8, 256, 256, 256, 4
NPIX = H * W                  # 65536 pixels per batch/core
FT = 4096                     # pixels per tile
NT = NPIX // FT               # 16 tiles
NBLK = FT // 128              # 32 pixel-blocks per tile
EPS = 1e-20
NCORES = 8

# Chebyshev seed constants for the NOT-trick reciprocal (see concourse/dve_ops.py)
RC0, RC1 = -0.23549792, 2.0017324

V_LO = np.float32(5.960464477539063e-08)  # 2^-24: smallest fp16 subnormal.
# f32 spacing in [0.5, 1) is 2^-24, so 1-u is an exact multiple of 2^-24 and
# fp16 subnormals encode the whole gumbel tail EXACTLY (requires the device
# datapath not to flush fp16 subnormal inputs - verified on HW).
V_HI = np.float32(0.99951171875)     # 1 - 2^-11: largest fp16 < 1

_cache: dict = {}
_div_op = None


def _get_div_op():
    """Register (once) a fused approximate-divide custom DVE op:
        out = Src1 * recip1nr(Src0)
    where recip1nr = bitcast-NOT seed + one Newton step (~0.2% rel err).
    Same seed stages as production RECIPROCAL_APPROX_FAST, minus one Newton
    pass, plus a trailing multiply; fits the 8-stage DVE pipeline."""
    global _div_op
    if _div_op is not None:
        return _div_op
    from concourse import dve_ops
    from concourse.dve_spec import Spec, Bin, AluOp, Src0, Src1, C0, C1, lower
    from concourse.dve_spec import _has_src1
    from concourse.dve_uop import DveOpSpec

    name = "DIV_RECIP1NR_ANT"
    for op in dve_ops.OPS:
        if op.name == name:
            _div_op = op
            return op

    _not_x = Bin(AluOp.BITWISE_NOT, Src0, Src0)
    _y0 = _not_x * C0
    _y1 = _y0 * (C1 - Src0 * _y0)
    body = _y1 * Src1

    def _ref(in0, in1, c0, c1, c2):
        in0 = np.asarray(in0, np.float32)
        not_x = (~in0.view(np.int32)).view(np.float32)
        y0 = not_x * np.float32(c0)
        y1 = y0 * (np.float32(c1) - in0 * y0)
        return (y1 * np.asarray(in1, np.float32)).astype(np.float32)

    spec = Spec(body=body, reference=_ref)
    row = max(dve_ops._SUB_OPCODE_FOR_NAME.values()) + 1
    assert row < 0x20
    dve_ops._SUB_OPCODE_FOR_NAME[name] = row
    shas = {}
    for ver in ("v3",):  # TRN2
        uops = lower(spec, ver=ver)
        shas[ver] = DveOpSpec(
            name=name, opcode=row, uops=uops, rd1_en=_has_src1(spec)
        ).sha(ver)
    op = dve_ops.DveOp(name, spec, subdim=False, uops_sha=shas)
    dve_ops.OPS.append(op)
    dve_ops.CUSTOM_DVE_SPECS[name] = spec
    _div_op = op
    return op


def _build(temp: float, repeat: int = 1):
    import concourse.mybir as mybir
    from concourse import bacc
    from concourse.tile import TileContext

    dt = mybir.dt
    AF = mybir.ActivationFunctionType
    div_op = _get_div_op()

    nc = bacc.Bacc("TRN2", target_bir_lowering=False, debug=False,
                   num_devices=NCORES)

    img_d = nc.dram_tensor("images", [NT, 2, 128, FT], dt.float16, kind="ExternalInput")
    noi_d = nc.dram_tensor("noise", [NT, 2, 128, FT], dt.float16, kind="ExternalInput")
    pal_d = nc.dram_tensor("pal", [128, 2, 5], dt.bfloat16, kind="ExternalInput")
    out_d = nc.dram_tensor("out", [NT, 128, NBLK * 4], dt.float32, kind="ExternalOutput")

    fast = (temp == 1.0)

    with TileContext(nc) as tc:
        with (
            tc.tile_pool(name="const", bufs=1) as cpool,
            tc.tile_pool(name="img", bufs=2) as ipool,
            tc.tile_pool(name="noi", bufs=2) as npool,
            tc.tile_pool(name="tln", bufs=2) as tpool,
            tc.tile_pool(name="xex", bufs=2) as xpool,
            tc.tile_pool(name="raw", bufs=2) as rpool,
            tc.tile_pool(name="outp", bufs=2) as opool,
            tc.tile_pool(name="acc", bufs=2, space="PSUM") as accpool,
        ):
            pal = cpool.tile([128, 2, 5], dt.bfloat16, tag="pal")
            nc.sync.dma_start(pal[:], pal_d[:])
            epsb = None
            if not fast:
                epsb = cpool.tile([128, 1], dt.float32, tag="epsb")
                nc.vector.memset(epsb[:], EPS)

            for _rep in range(repeat):
                for ti in range(NT):
                    img = ipool.tile([128, 2, FT], dt.float16)
                    noi = npool.tile([128, 2, FT], dt.float16)
                    for h in range(2):
                        nc.sync.dma_start(img[:, h, :], img_d[ti, h])
                        nc.sync.dma_start(noi[:, h, :], noi_d[ti, h])

                    x = xpool.tile([128, 2, FT], dt.bfloat16)
                    t = tpool.tile([128, 2, FT], dt.float32)
                    if fast:
                        # t = ln(1 - v) = ln(u)  (< 0)
                        nc.scalar.activation(t[:], noi[:], AF.Ln, scale=-1.0, bias=1.0)
                        # x = exp(img)
                        nc.scalar.activation(x[:], img[:], AF.Exp)
                        # x = x * approx(1/t)  (negative; sign cancels in ratio)
                        nc.vector._custom_dve(div_op, out=x[:], in0=t[:], in1=x[:],
                                              s0=RC0, s1=RC1)
                    else:
                        # general T: t = ln(u); G = ln(eps - t) = -gumbel;
                        # z = img - G; x = exp(z/T)
                        nc.scalar.activation(t[:], noi[:], AF.Ln, scale=-1.0, bias=1.0)
                        nc.scalar.activation(t[:], t[:], AF.Ln, scale=-1.0, bias=epsb[:])
                        nc.vector.tensor_sub(t[:], img[:], t[:])
                        nc.scalar.activation(x[:], t[:], AF.Exp, scale=1.0 / temp)

                    acc = accpool.tile([128, NBLK * 5], dt.float32)
                    for j in range(NBLK):
                        for h in range(2):
                            nc.tensor.matmul(
                                acc[:, j * 5:(j + 1) * 5],
                                x[:, h, j * 128:(j + 1) * 128],
                                pal[:, h, :],
                                start=(h == 0),
                                stop=(h == 1),
                            )

                    # epilogue: out_c = acc_c * approx(1/acc_4) per pixel-block
                    raw = rpool.tile([128, NBLK * 5], dt.float32)
                    nc.vector.tensor_copy(raw[:], acc[:])
                    rv = raw[:].rearrange("p (j c) -> p j c", c=5)
                    outf = opool.tile([128, NBLK * 4], dt.float32)
                    ov = outf[:].rearrange("p (j c) -> p j c", c=4)
                    for c in range(4):
                        nc.vector._custom_dve(div_op, out=ov[:, :, c],
                                              in0=rv[:, :, 4], in1=rv[:, :, c],
                                              s0=RC0, s1=RC1)
                    nc.sync.dma_start(out_d[ti], outf[:])

    nc.compile()
    return nc


def _get_nc(temp: float, repeat: int = 1):
    key = (temp, repeat)
    if key not in _cache:
        _cache[key] = _build(temp, repeat)
    return _cache[key]


def _to_kmajor(a16: np.ndarray) -> np.ndarray:
    """[NPIX, K] fp16 -> [NT, 2, 128, FT] contiguous k-major tiles."""
    # (ti*FT+f, h*128+p) -> [ti, h, p, f]
    return np.ascontiguousarray(
        a16.reshape(NT, FT, 2, 128).transpose(0, 2, 3, 1)
    )


def _make_in_maps(images, palettes, uniform_noise):
    in_maps = []
    for i in range(NCORES):
        img16 = images[i].reshape(NPIX, K).astype(np.float16)
        v = np.clip(
            np.float32(1.0) - uniform_noise[i].reshape(NPIX, K), V_LO, V_HI
        ).astype(np.float16)
        aug = np.concatenate(
            [palettes[i].astype(np.float32), np.ones((K, 1), np.float32)], axis=1
        )  # [256, 5]
        pal = np.ascontiguousarray(
            aug.reshape(2, 128, 5).transpose(1, 0, 2)
        ).astype(ml_dtypes.bfloat16)  # [128(k_lo), 2(k_hi), 5]
        in_maps.append(
            {
                "images": _to_kmajor(img16),
                "noise": _to_kmajor(v),
                "pal": pal,
            }
        )
    return in_maps


def _unshard(results):
    outs = []
    for i in range(NCORES):
        o = np.asarray(results[i]["out"], dtype=np.float32)  # [NT,128,NBLK*4]
        o = o.reshape(NT, 128, NBLK, 4).transpose(0, 2, 1, 3)  # [NT,NBLK,128,4]
        outs.append(o.reshape(NPIX, 4).reshape(H, W, 4))
    return np.stack(outs)  # [8, 256, 256, 4]


def kernel(**inputs) -> np.ndarray:
    from concourse.bass_utils import run_bass_kernel_spmd

    images = np.asarray(inputs["images"], dtype=np.float32)
    palettes = np.asarray(inputs["palettes"], dtype=np.float32)
    noise = np.asarray(inputs["uniform_noise"], dtype=np.float32)
    temp = float(np.asarray(inputs["temperature"]))

    nc = _get_nc(temp)
    in_maps = _make_in_maps(images, palettes, noise)
    res = run_bass_kernel_spmd(nc, in_maps, list(range(NCORES)))
    return _unshard(res.results)


# revision 11
# speedup vs baseline: 2.1730x; 2.1730x over previous
"""Gumbel-softmax palette quantization on 8 TRN2 NeuronCores.

Math (per batch b, pixel p, palette entry k, temperature T):
    gumbel = -ln(-ln(u + eps) + eps)
    probs  = softmax((img + gumbel) / T, axis=k)
    out    = probs @ palette                          # [pix, 4]

T == 1 fast path used here (setup_inputs always has T=1):
    exp(img + gumbel) = exp(img) / (-ln u)
and softmax ratios are invariant to a common per-pixel scale/sign, so with
    t = ln(1 - v)        # v = fp16(1-u) host-encoded, t = ln(u) < 0
    x = exp(img)         # ACT Exp, bf16
    e = x * approx(1/t)  # single fused custom-DVE op (NOT-seed + 1 Newton),
                         # e < 0; sign cancels in the final ratio
    out[p, c] = (e @ [pal|1])[p, c] / (e @ [pal|1])[p, 4]

Why v = 1-u: ln(u) for u->1 is where all the gumbel weight lives, and
fp16(u) destroys 1-u there. 1-u is exact in fp32 for u >= 0.5 (Sterbenz), so
fp16(1-u) keeps ~2^-11 relative precision on t in the tail. The device
computes t = Ln(-v + 1.0) via ACT's free affine - identical instruction cost.

Engine budget per core (16.7M elements):
    ACT  : Ln pass + Exp pass            ~221 us  <- bottleneck (1 elem/cyc/lane)
    DMA  : 2 x 32MiB fp16 in + 1MiB out  ~190 us
    DVE  : fused divide (1x) + epilogue  ~150 us
    PE   : 1024 matmuls [128k,128p]@[128k,5] ~70 us

Sharding: data-parallel over batch, 1 batch per core (b=8, 8 cores).

Layout: host pre-transposes img/noise to k-major [tile, khalf, 128, FT] so
the ACT Exp output is directly the matmul lhsT (k on partitions) - no
device transposes at all. Output written as [tile, 128, block, 4];
host untangles.
"""

import numpy as np
import ml_dtypes

B, H, W, K, C = 8, 256, 256, 256, 4
NPIX = H * W                  # 65536 pixels per batch/core
FT = 4096                     # pixels per tile
NT = NPIX // FT               # 16 tiles
NBLK = FT // 128              # 32 pixel-blocks per tile
EPS = 1e-20
NCORES = 8

# Chebyshev seed constants for the NOT-trick reciprocal (see concourse/dve_ops.py)
RC0, RC1 = -0.23549792, 2.0017324

V_LO = np.float32(5.960464477539063e-08)  # 2^-24: smallest fp16 subnormal.
# f32 spacing in [0.5, 1) is 2^-24, so 1-u is an exact multiple of 2^-24 and
# fp16 subnormals encode the whole gumbel tail EXACTLY (requires the device
# datapath not to flush fp16 subnormal inputs - verified on HW).
V_HI = np.float32(0.99951171875)     # 1 - 2^-11: largest fp16 < 1

_cache: dict = {}
_div_op = None


def _get_div_op():
    """Register (once) a fused approximate-divide custom DVE op:
        out = Src1 * recip1nr(Src0)
    where recip1nr = bitcast-NOT seed + one Newton step (~0.2% rel err).
    Same seed stages as production RECIPROCAL_APPROX_FAST, minus one Newton
    pass, plus a trailing multiply; fits the 8-stage DVE pipeline."""
    global _div_op
    if _div_op is not None:
        return _div_op
    from concourse import dve_ops
    from concourse.dve_spec import Spec, Bin, AluOp, Src0, Src1, C0, C1, lower
    from concourse.dve_spec import _has_src1
    from concourse.dve_uop import DveOpSpec

    name = "DIV_RECIP1NR_ANT"
    for op in dve_ops.OPS:
        if op.name == name:
            _div_op = op
            return op

    _not_x = Bin(AluOp.BITWISE_NOT, Src0, Src0)
    _y0 = _not_x * C0
    _y1 = _y0 * (C1 - Src0 * _y0)
    body = _y1 * Src1

    def _ref(in0, in1, c0, c1, c2):
        in0 = np.asarray(in0, np.float32)
        not_x = (~in0.view(np.int32)).view(np.float32)
        y0 = not_x * np.float32(c0)
        y1 = y0 * (np.float32(c1) - in0 * y0)
        return (y1 * np.asarray(in1, np.float32)).astype(np.float32)

    spec = Spec(body=body, reference=_ref)
    row = max(dve_ops._SUB_OPCODE_FOR_NAME.values()) + 1
    assert row < 0x20
    dve_ops._SUB_OPCODE_FOR_NAME[name] = row
    shas = {}
    for ver in ("v3",):  # TRN2
        uops = lower(spec, ver=ver)
        shas[ver] = DveOpSpec(
            name=name, opcode=row, uops=uops, rd1_en=_has_src1(spec)
        ).sha(ver)
    op = dve_ops.DveOp(name, spec, subdim=False, uops_sha=shas)
    dve_ops.OPS.append(op)
    dve_ops.CUSTOM_DVE_SPECS[name] = spec
    _div_op = op
    return op


def _build(temp: float, repeat: int = 1):
    import concourse.mybir as mybir
    from concourse import bacc
    from concourse.tile import TileContext

    dt = mybir.dt
    AF = mybir.ActivationFunctionType
    div_op = _get_div_op()

    nc = bacc.Bacc("TRN2", target_bir_lowering=False, debug=False,
                   num_devices=NCORES)

    img_d = nc.dram_tensor("images", [NT, 2, 128, FT], dt.float16, kind="ExternalInput")
    noi_d = nc.dram_tensor("noise", [NT, 2, 128, FT], dt.float16, kind="ExternalInput")
    pal_d = nc.dram_tensor("pal", [128, 2, 5], dt.bfloat16, kind="ExternalInput")
    out_d = nc.dram_tensor("out", [NT, 128, NBLK * 4], dt.float32, kind="ExternalOutput")

    fast = (temp == 1.0)

    with TileContext(nc) as tc:
        with (
            tc.tile_pool(name="const", bufs=1) as cpool,
            tc.tile_pool(name="img", bufs=3) as ipool,
            tc.tile_pool(name="noi", bufs=3) as npool,
            tc.tile_pool(name="tln", bufs=2) as tpool,
            tc.tile_pool(name="xex", bufs=2) as xpool,
            tc.tile_pool(name="raw", bufs=2) as rpool,
            tc.tile_pool(name="outp", bufs=2) as opool,
            tc.tile_pool(name="acc", bufs=2, space="PSUM") as accpool,
        ):
            pal = cpool.tile([128, 2, 5], dt.bfloat16, tag="pal")
            nc.sync.dma_start(pal[:], pal_d[:])
            epsb = None
            if not fast:
                epsb = cpool.tile([128, 1], dt.float32, tag="epsb")
                nc.vector.memset(epsb[:], EPS)

            for _rep in range(repeat):
                for ti in range(NT):
                    img = ipool.tile([128, 2, FT], dt.float16)
                    noi = npool.tile([128, 2, FT], dt.float16)
                    for h in range(2):
                        nc.sync.dma_start(img[:, h, :], img_d[ti, h])
                        nc.sync.dma_start(noi[:, h, :], noi_d[ti, h])

                    x = xpool.tile([128, 2, FT], dt.bfloat16)
                    t = tpool.tile([128, 2, FT], dt.float32)
                    if fast:
                        # t = ln(1 - v) = ln(u)  (< 0)
                        nc.scalar.activation(t[:], noi[:], AF.Ln, scale=-1.0, bias=1.0)
                        # x = exp(img)
                        nc.scalar.activation(x[:], img[:], AF.Exp)
                        # x = x * approx(1/t)  (negative; sign cancels in ratio)
                        nc.vector._custom_dve(div_op, out=x[:], in0=t[:], in1=x[:],
                                              s0=RC0, s1=RC1)
                    else:
                        # general T: t = ln(u); G = ln(eps - t) = -gumbel;
                        # z = img - G; x = exp(z/T)
                        nc.scalar.activation(t[:], noi[:], AF.Ln, scale=-1.0, bias=1.0)
                        nc.scalar.activation(t[:], t[:], AF.Ln, scale=-1.0, bias=epsb[:])
                        nc.vector.tensor_sub(t[:], img[:], t[:])
                        nc.scalar.activation(x[:], t[:], AF.Exp, scale=1.0 / temp)

                    acc = accpool.tile([128, NBLK * 5], dt.float32)
                    for j in range(NBLK):
                        for h in range(2):
                            nc.tensor.matmul(
                                acc[:, j * 5:(j + 1) * 5],
                                x[:, h, j * 128:(j + 1) * 128],
                                pal[:, h, :],
                                start=(h == 0),
                                stop=(h == 1),
                            )

                    # epilogue: out_c = acc_c * approx(1/acc_4) per pixel-block
                    raw = rpool.tile([128, NBLK * 5], dt.float32)
                    nc.vector.tensor_copy(raw[:], acc[:])
                    rv = raw[:].rearrange("p (j c) -> p j c", c=5)
                    outf = opool.tile([128, NBLK * 4], dt.float32)
                    ov = outf[:].rearrange("p (j c) -> p j c", c=4)
                    for c in range(4):
                        nc.vector._custom_dve(div_op, out=ov[:, :, c],
                                              in0=rv[:, :, 4], in1=rv[:, :, c],
                                              s0=RC0, s1=RC1)
                    nc.sync.dma_start(out_d[ti], outf[:])

    nc.compile()
    return nc


def _get_nc(temp: float, repeat: int = 1):
    key = (temp, repeat)
    if key not in _cache:
        _cache[key] = _build(temp, repeat)
    return _cache[key]


def _to_kmajor(a16: np.ndarray) -> np.ndarray:
    """[NPIX, K] fp16 -> [NT, 2, 128, FT] contiguous k-major tiles."""
    # (ti*FT+f, h*128+p) -> [ti, h, p, f]
    return np.ascontiguousarray(
        a16.reshape(NT, FT, 2, 128).transpose(0, 2, 3, 1)
    )


def _make_in_maps(images, palettes, uniform_noise):
    in_maps = []
    for i in range(NCORES):
        img16 = images[i].reshape(NPIX, K).astype(np.float16)
        v = np.clip(
            np.float32(1.0) - uniform_noise[i].reshape(NPIX, K), V_LO, V_HI
        ).astype(np.float16)
        aug = np.concatenate(
            [palettes[i].astype(np.float32), np.ones((K, 1), np.float32)], axis=1
        )  # [256, 5]
        pal = np.ascontiguousarray(
            aug.reshape(2, 128, 5).transpose(1, 0, 2)
        ).astype(ml_dtypes.bfloat16)  # [128(k_lo), 2(k_hi), 5]
        in_maps.append(
            {
                "images": _to_kmajor(img16),
                "noise": _to_kmajor(v),
                "pal": pal,
            }
        )
    return in_maps


def _unshard(results):
    outs = []
    for i in range(NCORES):
        o = np.asarray(results[i]["out"], dtype=np.float32)  # [NT,128,NBLK*4]
        o = o.reshape(NT, 128, NBLK, 4).transpose(0, 2, 1, 3)  # [NT,NBLK,128,4]
        outs.append(o.reshape(NPIX, 4).reshape(H, W, 4))
    return np.stack(outs)  # [8, 256, 256, 4]


def kernel(**inputs) -> np.ndarray:
    from concourse.bass_utils import run_bass_kernel_spmd

    images = np.asarray(inputs["images"], dtype=np.float32)
    palettes = np.asarray(inputs["palettes"], dtype=np.float32)
    noise = np.asarray(inputs["uniform_noise"], dtype=np.float32)
    temp = float(np.asarray(inputs["temperature"]))

    nc = _get_nc(temp)
    in_maps = _make_in_maps(images, palettes, noise)
    res = run_bass_kernel_spmd(nc, in_maps, list(range(NCORES)))
    return _unshard(res.results)


# revision 12
# speedup vs baseline: 15.1572x; 6.9751x over previous
"""Gumbel-softmax palette quantization on 8 TRN2 NeuronCores.

Math (per batch b, pixel p, palette entry k, temperature T):
    gumbel = -ln(-ln(u + eps) + eps)
    probs  = softmax((img + gumbel) / T, axis=k)
    out    = probs @ palette                          # [pix, 4]

T == 1 fast path used here (setup_inputs always has T=1):
    exp(img + gumbel) = exp(img) / (-ln u)
and softmax ratios are invariant to a common per-pixel scale/sign, so with
    t = ln(1 - v)        # v = fp16(1-u) host-encoded, t = ln(u) < 0
    x = exp(img)         # ACT Exp, bf16
    e = x * approx(1/t)  # single fused custom-DVE op (NOT-seed + 1 Newton),
                         # e < 0; sign cancels in the final ratio
    out[p, c] = (e @ [pal|1])[p, c] / (e @ [pal|1])[p, 4]

Why v = 1-u: ln(u) for u->1 is where all the gumbel weight lives, and
fp16(u) destroys 1-u there. 1-u is exact in fp32 for u >= 0.5 (Sterbenz), so
fp16(1-u) keeps ~2^-11 relative precision on t in the tail. The device
computes t = Ln(-v + 1.0) via ACT's free affine - identical instruction cost.

Engine budget per core (16.7M elements):
    ACT  : Ln pass + Exp pass            ~221 us  <- bottleneck (1 elem/cyc/lane)
    DMA  : 2 x 32MiB fp16 in + 1MiB out  ~190 us
    DVE  : fused divide (1x) + epilogue  ~150 us
    PE   : 1024 matmuls [128k,128p]@[128k,5] ~70 us

Sharding: data-parallel over batch, 1 batch per core (b=8, 8 cores).

Layout: host pre-transposes img/noise to k-major [tile, khalf, 128, FT] so
the ACT Exp output is directly the matmul lhsT (k on partitions) - no
device transposes at all. Output written as [tile, 128, block, 4];
host untangles.
"""

import numpy as np
import ml_dtypes

B, H, W, K, C = 8, 256, 256, 256, 4
NPIX = H * W                  # 65536 pixels per batch/core
FT = 4096                     # pixels per tile
NT = NPIX // FT               # 16 tiles
NBLK = FT // 128              # 32 pixel-blocks per tile
EPS = 1e-20
NCORES = 8

# Chebyshev seed constants for the NOT-trick reciprocal (see concourse/dve_ops.py)
RC0, RC1 = -0.23549792, 2.0017324

V_LO = np.float32(5.960464477539063e-08)  # 2^-24: smallest fp16 subnormal.
# f32 spacing in [0.5, 1) is 2^-24, so 1-u is an exact multiple of 2^-24 and
# fp16 subnormals encode the whole gumbel tail EXACTLY (requires the device
# datapath not to flush fp16 subnormal inputs - verified on HW).
V_HI = np.float32(0.99951171875)     # 1 - 2^-11: largest fp16 < 1

_cache: dict = {}
_div_op = None


def _get_div_op():
    """Register (once) a fused approximate-divide custom DVE op:
        out = Src1 * recip1nr(Src0)
    where recip1nr = bitcast-NOT seed + one Newton step (~0.2% rel err).
    Same seed stages as production RECIPROCAL_APPROX_FAST, minus one Newton
    pass, plus a trailing multiply; fits the 8-stage DVE pipeline."""
    global _div_op
    if _div_op is not None:
        return _div_op
    from concourse import dve_ops
    from concourse.dve_spec import Spec, Bin, AluOp, Src0, Src1, C0, C1, lower
    from concourse.dve_spec import _has_src1
    from concourse.dve_uop import DveOpSpec

    name = "DIV_RECIP1NR_ANT"
    for op in dve_ops.OPS:
        if op.name == name:
            _div_op = op
            return op

    _not_x = Bin(AluOp.BITWISE_NOT, Src0, Src0)
    _y0 = _not_x * C0
    _y1 = _y0 * (C1 - Src0 * _y0)
    body = _y1 * Src1

    def _ref(in0, in1, c0, c1, c2):
        in0 = np.asarray(in0, np.float32)
        not_x = (~in0.view(np.int32)).view(np.float32)
        y0 = not_x * np.float32(c0)
        y1 = y0 * (np.float32(c1) - in0 * y0)
        return (y1 * np.asarray(in1, np.float32)).astype(np.float32)

    spec = Spec(body=body, reference=_ref)
    row = max(dve_ops._SUB_OPCODE_FOR_NAME.values()) + 1
    assert row < 0x20
    dve_ops._SUB_OPCODE_FOR_NAME[name] = row
    shas = {}
    for ver in ("v3",):  # TRN2
        uops = lower(spec, ver=ver)
        shas[ver] = DveOpSpec(
            name=name, opcode=row, uops=uops, rd1_en=_has_src1(spec)
        ).sha(ver)
    op = dve_ops.DveOp(name, spec, subdim=False, uops_sha=shas)
    dve_ops.OPS.append(op)
    dve_ops.CUSTOM_DVE_SPECS[name] = spec
    _div_op = op
    return op


def _build(temp: float, repeat: int = 1):
    import concourse.mybir as mybir
    from concourse import bacc
    from concourse.tile import TileContext

    dt = mybir.dt
    AF = mybir.ActivationFunctionType
    div_op = _get_div_op()

    nc = bacc.Bacc("TRN2", target_bir_lowering=False, debug=False,
                   num_devices=NCORES)

    img_d = nc.dram_tensor("images", [NT, 2, 128, FT], dt.float16, kind="ExternalInput")
    noi_d = nc.dram_tensor("noise", [NT, 2, 128, FT], dt.float16, kind="ExternalInput")
    pal_d = nc.dram_tensor("pal", [128, 2, 5], dt.bfloat16, kind="ExternalInput")
    out_d = nc.dram_tensor("out", [NT, 128, NBLK * 4], dt.float32, kind="ExternalOutput")

    fast = (temp == 1.0)

    with TileContext(nc) as tc:
        with (
            tc.tile_pool(name="const", bufs=1) as cpool,
            tc.tile_pool(name="img", bufs=2) as ipool,
            tc.tile_pool(name="noi", bufs=2) as npool,
            tc.tile_pool(name="tln", bufs=2) as tpool,
            tc.tile_pool(name="xex", bufs=2) as xpool,
            tc.tile_pool(name="raw", bufs=2) as rpool,
            tc.tile_pool(name="outp", bufs=2) as opool,
            tc.tile_pool(name="acc", bufs=2, space="PSUM") as accpool,
        ):
            pal = cpool.tile([128, 2, 5], dt.bfloat16, tag="pal")
            nc.sync.dma_start(pal[:], pal_d[:])
            epsb = None
            if not fast:
                epsb = cpool.tile([128, 1], dt.float32, tag="epsb")
                nc.vector.memset(epsb[:], EPS)

            for _rep in range(repeat):
                for ti in range(NT):
                    img = ipool.tile([128, 2, FT], dt.float16)
                    noi = npool.tile([128, 2, FT], dt.float16)
                    for h in range(2):
                        nc.sync.dma_start(img[:, h, :], img_d[ti, h])
                        nc.sync.dma_start(noi[:, h, :], noi_d[ti, h])

                    x = xpool.tile([128, 2, FT], dt.bfloat16)
                    t = tpool.tile([128, 2, FT], dt.float32)
                    if fast:
                        # t = ln(1 - v) = ln(u)  (< 0)
                        nc.scalar.activation(t[:], noi[:], AF.Ln, scale=-1.0, bias=1.0)
                        # x = exp(img)
                        nc.scalar.activation(x[:], img[:], AF.Exp)
                        # x = x * approx(1/t)  (negative; sign cancels in ratio)
                        nc.vector._custom_dve(div_op, out=x[:], in0=t[:], in1=x[:],
                                              s0=RC0, s1=RC1)
                    else:
                        # general T: t = ln(u); G = ln(eps - t) = -gumbel;
                        # z = img - G; x = exp(z/T)
                        nc.scalar.activation(t[:], noi[:], AF.Ln, scale=-1.0, bias=1.0)
                        nc.scalar.activation(t[:], t[:], AF.Ln, scale=-1.0, bias=epsb[:])
                        nc.vector.tensor_sub(t[:], img[:], t[:])
                        nc.scalar.activation(x[:], t[:], AF.Exp, scale=1.0 / temp)

                    acc = accpool.tile([128, NBLK * 5], dt.float32)
                    for j in range(NBLK):
                        for h in range(2):
                            nc.tensor.matmul(
                                acc[:, j * 5:(j + 1) * 5],
                                x[:, h, j * 128:(j + 1) * 128],
                                pal[:, h, :],
                                start=(h == 0),
                                stop=(h == 1),
                            )

                    # epilogue: out_c = acc_c * approx(1/acc_4) per pixel-block
                    raw = rpool.tile([128, NBLK * 5], dt.float32)
                    nc.vector.tensor_copy(raw[:], acc[:])
                    rv = raw[:].rearrange("p (j c) -> p j c", c=5)
                    outf = opool.tile([128, NBLK * 4], dt.float32)
                    ov = outf[:].rearrange("p (j c) -> p j c", c=4)
                    for c in range(4):
                        nc.vector._custom_dve(div_op, out=ov[:, :, c],
                                              in0=rv[:, :, 4], in1=rv[:, :, c],
                                              s0=RC0, s1=RC1)
                    nc.sync.dma_start(out_d[ti], outf[:])

    nc.compile()
    return nc


def _get_nc(temp: float, repeat: int = 1):
    key = (temp, repeat)
    if key not in _cache:
        _cache[key] = _build(temp, repeat)
    return _cache[key]


def _to_kmajor(a16: np.ndarray) -> np.ndarray:
    """[NPIX, K] fp16 -> [NT, 2, 128, FT] contiguous k-major tiles."""
    # (ti*FT+f, h*128+p) -> [ti, h, p, f]
    return np.ascontiguousarray(
        a16.reshape(NT, FT, 2, 128).transpose(0, 2, 3, 1)
    )


def _make_in_maps(images, palettes, uniform_noise):
    in_maps = []
    for i in range(NCORES):
        img16 = images[i].reshape(NPIX, K).astype(np.float16)
        v = np.clip(
            np.float32(1.0) - uniform_noise[i].reshape(NPIX, K), V_LO, V_HI
        ).astype(np.float16)
        aug = np.concatenate(
            [palettes[i].astype(np.float32), np.ones((K, 1), np.float32)], axis=1
        )  # [256, 5]
        pal = np.ascontiguousarray(
            aug.reshape(2, 128, 5).transpose(1, 0, 2)
        ).astype(ml_dtypes.bfloat16)  # [128(k_lo), 2(k_hi), 5]
        in_maps.append(
            {
                "images": _to_kmajor(img16),
                "noise": _to_kmajor(v),
                "pal": pal,
            }
        )
    return in_maps


def _unshard(results):
    outs = []
    for i in range(NCORES):
        o = np.asarray(results[i]["out"], dtype=np.float32)  # [NT,128,NBLK*4]
        o = o.reshape(NT, 128, NBLK, 4).transpose(0, 2, 1, 3)  # [NT,NBLK,128,4]
        outs.append(o.reshape(NPIX, 4).reshape(H, W, 4))
    return np.stack(outs)  # [8, 256, 256, 4]


def kernel(**inputs) -> np.ndarray:
    from concourse.bass_utils import run_bass_kernel_spmd

    images = np.asarray(inputs["images"], dtype=np.float32)
    palettes = np.asarray(inputs["palettes"], dtype=np.float32)
    noise = np.asarray(inputs["uniform_noise"], dtype=np.float32)
    temp = float(np.asarray(inputs["temperature"]))

    nc = _get_nc(temp)
    in_maps = _make_in_maps(images, palettes, noise)
    res = run_bass_kernel_spmd(nc, in_maps, list(range(NCORES)))
    return _unshard(res.results)
